# revision 3
# baseline (speedup 1.0000x reference)
"""2-layer GCN with residual (GCNResnet) on 8 Trainium2 NeuronCores.

Strategy (graph/data parallel per the sharding hint; v3):
- Nodes padded to N_pad = 8*BPC*128, sharded contiguously: core c owns BPC
  blocks of 128 destination nodes; edges assigned to the core owning their
  dst. Weights replicated.
- Key algebra: GCN aggregation commutes with the linear transform,
  out = A_hat @ (x @ W) = (A_hat @ x) @ W. So each layer gathers *input*
  rows (64 wide) and aggregates BEFORE applying the weight matrix — no
  full-graph h1 table is ever materialized (the old phase 0 is gone).
- The symmetric norm dinv[src]*dinv[dst] factors completely out of the
  per-edge selection: dinv[src] pre-scales the gather-table rows (x*dinv on
  host; h2*dinv on device), dinv[dst] post-scales each dst block. The
  per-edge-tile selection matrices S_t[e, loc] = (dst_local[e]==loc) are
  therefore BINARY -> precomputed host-side as exact fp8 tiles and kept
  SBUF-resident (~16MB), shared by both layers. This removes the 2x973
  DVE tensor_scalar builds that dominated the old kernel (1.3ms of DVE).
- Per dst block b: agg[loc, f] = sum_t S_t^T @ G_t (PE, fp8 lhsT x bf16
  rhs, PSUM accum). Layer 1 then: scale by dinv_c -> transpose via
  identity matmul -> y1T = W1^T @ aggT -> lrelu(+b1) -> h2 = y1 @ W2 ->
  scale by dinv_c (this is the L2 table's src pre-scale) -> h2sh shard.
- One AllGather assembles h2full [N_pad, 128] (rows padded to 256B for the
  gather-element-size floor; upper 64 cols junk, never read). Layer 2
  aggregates the same way (same indices, same S tiles since h2full keeps
  the natural node order), adds x + b2 residual.
- dma_gather indices are int16 -> tables addressed in two halves.
"""

import sys

sys.path.insert(0, "/opt/trn_rl_repo")

import numpy as np
import ml_dtypes

import concourse.bacc as bacc
import concourse.bass as bass
import concourse.mybir as mybir
import concourse.tile as tile
from concourse.bass_utils import run_bass_kernel_spmd

P = 128
NCORES = 8

BF16 = mybir.dt.bfloat16
F32 = mybir.dt.float32
I16 = mybir.dt.int16
FP8 = mybir.dt.float8e4

NEG_SLOPE = 0.01


# ----------------------------------------------------------------- host prep
def _preprocess(edge_index, n, grp):
    """Bucket edges by (dst block, src half), pad to 128-multiples with sizes
    shared across cores, and build per-core index/metadata + binary S tiles."""
    bpc = -(-n // (NCORES * P))  # blocks per core
    nblk = NCORES * bpc
    n_pad = nblk * P
    half = (nblk // 2) * P
    assert half <= 32768 and (n_pad - half) <= 32768

    src = np.concatenate([edge_index[0], np.arange(n, dtype=np.int64)]).astype(np.int64)
    dst = np.concatenate([edge_index[1], np.arange(n, dtype=np.int64)]).astype(np.int64)
    e_all = src.shape[0]

    deg = np.bincount(dst, minlength=n).astype(np.float32)
    dinv = np.where(deg > 0, 1.0 / np.sqrt(np.maximum(deg, 1e-12)), 0.0).astype(
        np.float32
    )

    blk = (dst // P).astype(np.int64)
    hlf = (src >= half).astype(np.int64)
    key = blk * 2 + hlf
    counts = np.bincount(key, minlength=2 * nblk)
    cnt = counts.reshape(nblk, 2)
    cnt_cs = cnt.reshape(NCORES, bpc, 2)

    tlo = np.ceil(cnt_cs[:, :, 0].max(axis=0) / P).astype(np.int64)  # [bpc]
    thi = np.ceil(cnt_cs[:, :, 1].max(axis=0) / P).astype(np.int64)

    groups = []
    s = 0
    while s < bpc:
        e = min(s + grp, bpc)
        groups.append(list(range(s, e)))
        s = e

    slot_lo_t0 = np.zeros(bpc, np.int64)
    slot_hi_t0 = np.zeros(bpc, np.int64)
    gathers = []  # (half, tile0, ntiles, slots)
    t = 0
    for g in groups:
        g_lo0 = t
        for s_ in g:
            slot_lo_t0[s_] = t
            t += tlo[s_]
        if t > g_lo0:
            gathers.append((0, g_lo0, t - g_lo0, list(g)))
        g_hi0 = t
        for s_ in g:
            slot_hi_t0[s_] = t
            t += thi[s_]
        if t > g_hi0:
            gathers.append((1, g_hi0, t - g_hi0, list(g)))
    tt = int(t)
    tote = tt * P

    base = np.zeros((bpc, 2), np.int64)
    for s_ in range(bpc):
        base[s_, 0] = slot_lo_t0[s_] * P
        base[s_, 1] = slot_hi_t0[s_] * P

    order = np.argsort(key, kind="stable")
    sk = key[order]
    seg_start = np.concatenate([[0], np.cumsum(counts)[:-1]])
    rank_sorted = np.arange(e_all, dtype=np.int64) - seg_start[sk]
    rank = np.empty(e_all, np.int64)
    rank[order] = rank_sorted

    core = blk // bpc
    slot = blk % bpc
    padpos = base[slot, hlf] + rank

    idx_arr = np.zeros((NCORES, tote), np.int16)
    dl_arr = np.full((NCORES, tote), -1, np.int16)
    idx_arr[core, padpos] = (src - hlf * half).astype(np.int16)
    dl_arr[core, padpos] = (dst % P).astype(np.int16)

    # wrapped int16 index layout: idx j of a gather -> [j%16 (+16r), col0+j//16]
    idx16 = np.zeros((NCORES, P, tote // 16), np.int16)
    for hf, t0, nt, _slots in gathers:
        e0, e1 = t0 * P, (t0 + nt) * P
        seg = idx_arr[:, e0:e1].reshape(NCORES, (e1 - e0) // 16, 16)
        wrapped = seg.transpose(0, 2, 1)
        idx16[:, :, e0 // 16 : e1 // 16] = np.tile(wrapped, (1, 8, 1))

    # binary selection tiles: s_all[c, e, t, loc] = (dl[c, t*128+e] == loc)
    dl_t = dl_arr.reshape(NCORES, tt, P)  # [c, t, e]
    onehot = dl_t[:, :, :, None] == np.arange(P, dtype=np.int16)  # [c, t, e, loc]
    s_all = onehot.transpose(0, 2, 1, 3).astype(ml_dtypes.float8_e4m3)  # [c,e,t,loc]
    s_all = np.ascontiguousarray(s_all)

    slots = [
        dict(lo0=int(slot_lo_t0[s_]), nlo=int(tlo[s_]), hi0=int(slot_hi_t0[s_]),
             nhi=int(thi[s_]))
        for s_ in range(bpc)
    ]

    dinv_pad = np.zeros(n_pad, np.float32)
    dinv_pad[:n] = dinv
    dinv_c = dinv_pad.reshape(NCORES, bpc, P).transpose(0, 2, 1).copy()  # [c,128,bpc]

    plan = dict(n=n, n_pad=n_pad, bpc=bpc, nblk=nblk, half=half, tt=tt,
                gathers=gathers, slots=slots)
    percore = dict(idx16=idx16, s_all=s_all, dinv_c=dinv_c)
    return plan, percore, dinv_pad


# ------------------------------------------------------------ program build
def _build_program(plan, feat, hid, repeat=1, gbufs=8):
    n_pad, bpc, nblk, half, tt = (
        plan["n_pad"], plan["bpc"], plan["nblk"], plan["half"], plan["tt"]
    )
    nsh = bpc * P
    assert feat <= P and hid == P

    nc = bacc.Bacc("TRN2", target_bir_lowering=False, debug=False,
                   num_devices=NCORES, num_swdge_queues=4)

    # inputs (replicated)
    xs = nc.dram_tensor("xs", [n_pad, P], BF16, kind="ExternalInput")  # x*dinv rows
    W1 = nc.dram_tensor("W1", [feat, hid], BF16, kind="ExternalInput")
    b1 = nc.dram_tensor("b1", [P, 1], F32, kind="ExternalInput")
    W2 = nc.dram_tensor("W2", [hid, feat], BF16, kind="ExternalInput")
    ident = nc.dram_tensor("ident", [P, P], BF16, kind="ExternalInput")
    # inputs (per core)
    idx16 = nc.dram_tensor("idx16", [P, tt * P // 16], I16, kind="ExternalInput")
    s_all = nc.dram_tensor("s_all", [P, tt, P], FP8, kind="ExternalInput")
    dinv_c = nc.dram_tensor("dinv_c", [P, bpc], F32, kind="ExternalInput")
    xb2 = nc.dram_tensor("xb2", [nsh, feat], F32, kind="ExternalInput")
    # output
    out = nc.dram_tensor("out", [nsh, feat], F32, kind="ExternalOutput")

    # internal DRAM
    h2sh = nc.dram_tensor("h2sh", [nsh, P], BF16)
    h2full = nc.dram_tensor("h2full", [n_pad, P], BF16, addr_space="Shared")

    outr = out.rearrange("(n p) d -> p n d", p=P)  # [128, bpc, feat]
    h2shr = h2sh.rearrange("(n p) d -> p n d", p=P)  # [128, bpc, 128]

    gathers = plan["gathers"]
    slots = plan["slots"]
    max_g = max(g[2] for g in gathers)

    with tile.TileContext(nc) as tc:
        with (
            tc.tile_pool(name="const", bufs=1) as cpool,
            tc.tile_pool(name="gt", bufs=gbufs) as gpool,
            tc.tile_pool(name="stage", bufs=2) as stpool,
            tc.tile_pool(name="work", bufs=3) as wpool,
            tc.tile_pool(name="psAgg", bufs=3, space="PSUM") as psAgg,
            tc.tile_pool(name="psT", bufs=1, space="PSUM") as psTp,
            tc.tile_pool(name="psY", bufs=2, space="PSUM") as psYp,
            tc.tile_pool(name="psH", bufs=2, space="PSUM") as psHp,
        ):
            # ---- resident constants / metadata
            W1_sb = cpool.tile([feat, hid], BF16)
            nc.sync.dma_start(out=W1_sb[:], in_=W1[:])
            W2_sb = cpool.tile([hid, feat], BF16)
            nc.sync.dma_start(out=W2_sb[:], in_=W2[:])
            b1_sb = cpool.tile([P, 1], F32)
            nc.sync.dma_start(out=b1_sb[:], in_=b1[:])
            id_sb = cpool.tile([P, P], BF16)
            nc.sync.dma_start(out=id_sb[:], in_=ident[:])
            dinvc_sb = cpool.tile([P, bpc], F32)
            nc.sync.dma_start(out=dinvc_sb[:], in_=dinv_c[:])
            idx_sb = cpool.tile([P, tt * P // 16], I16)
            nc.sync.dma_start(out=idx_sb[:], in_=idx16[:])
            xb2_sb = cpool.tile([P, bpc, feat], F32)
            nc.sync.dma_start(
                out=xb2_sb[:], in_=xb2.rearrange("(n p) d -> p n d", p=P)[:]
            )
            s_sb = cpool.tile([P, tt, P], FP8)
            nc.sync.dma_start(out=s_sb[:], in_=s_all[:])

            def aggregate(layer, table, out_cb):
                """gather + S-matmul aggregation over this core's blocks."""
                gt = {}
                for gq, (hf, t0, ntl, _slots_g) in enumerate(gathers):
                    g = gpool.tile([P, max_g, P], BF16, tag="g",
                                   name=f"g{layer}_{hf}_{t0}")
                    nc.gpsimd.dma_gather(
                        out_ap=g[:, :ntl, :],
                        in_ap=table[hf * half : hf * half + half, :],
                        idxs_ap=idx_sb[:, t0 * 8 : (t0 + ntl) * 8],
                        num_idxs=ntl * P,
                        num_idxs_reg=ntl * P,
                        elem_size=P,
                        single_packet=False,
                        queue_num=gq % 4,
                    )
                    gt[(hf, t0)] = g

                for b, sl in enumerate(slots):
                    runs = []
                    if sl["nlo"]:
                        runs.append((0, sl["lo0"], sl["nlo"]))
                    if sl["nhi"]:
                        runs.append((1, sl["hi0"], sl["nhi"]))
                    nt_b = sum(r[2] for r in runs)
                    assert nt_b > 0
                    ps = psAgg.tile([P, feat], F32, space="PSUM", tag="ps_agg")
                    k = 0
                    for hf, t0, ntl in runs:
                        own = None
                        for hf2, gt0, gnt, _s in gathers:
                            if hf2 == hf and gt0 <= t0 and t0 + ntl <= gt0 + gnt:
                                own = (hf2, gt0)
                                break
                        g = gt[own]
                        goff = t0 - own[1]
                        for i in range(ntl):
                            tcol = t0 + i
                            nc.tensor.matmul(
                                out=ps[:],
                                lhsT=s_sb[:, tcol, :],
                                rhs=g[:, goff + i, :feat],
                                start=(k == 0), stop=(k == nt_b - 1),
                            )
                            k += 1
                    out_cb(b, ps)

            def layers():
                h2_stage = {}

                def l1_out(b, ps):
                    # dinv[dst] post-scale; -> bf16
                    agg_s = wpool.tile([P, feat], BF16, tag="agg_s")
                    nc.vector.tensor_scalar(
                        out=agg_s[:], in0=ps[:],
                        scalar1=dinvc_sb[:, b : b + 1], scalar2=None,
                        op0=mybir.AluOpType.mult,
                    )
                    # transpose [loc, feat] -> [feat, loc] via identity matmul
                    psT = psTp.tile([feat, P], F32, space="PSUM", tag="ps_t")
                    nc.tensor.matmul(out=psT[:], lhsT=agg_s[:], rhs=id_sb[:],
                                     start=True, stop=True)
                    aggT_s = wpool.tile([feat, P], BF16, tag="aggT_s")
                    nc.scalar.activation(
                        out=aggT_s[:], in_=psT[:],
                        func=mybir.ActivationFunctionType.Copy,
                    )
                    psY = psYp.tile([P, P], F32, space="PSUM", tag="ps_y")
                    nc.tensor.matmul(out=psY[:], lhsT=W1_sb[:], rhs=aggT_s[:],
                                     start=True, stop=True)
                    a1 = wpool.tile([P, P], BF16, tag="a1")
                    nc.scalar.activation(
                        out=a1[:], in_=psY[:],
                        func=mybir.ActivationFunctionType.Lrelu,
                        bias=b1_sb[:, :1], scale=1.0, alpha=NEG_SLOPE,
                    )
                    psH = psHp.tile([P, feat], F32, space="PSUM", tag="ps_h")
                    nc.tensor.matmul(out=psH[:], lhsT=a1[:], rhs=W2_sb[:],
                                     start=True, stop=True)
                    j0 = (b // 8) * 8
                    if j0 not in h2_stage:
                        h2_stage[j0] = stpool.tile([P, 8, feat], BF16, tag="h2st",
                                                   name=f"h2st{j0}")
                    st = h2_stage[j0]
                    # dinv pre-scale of the L2 gather table rows
                    nc.vector.tensor_scalar(
                        out=st[:, b - j0, :], in0=psH[:],
                        scalar1=dinvc_sb[:, b : b + 1], scalar2=None,
                        op0=mybir.AluOpType.mult,
                    )
                    if b == min(j0 + 7, bpc - 1):
                        nc.sync.dma_start(out=h2shr[:, j0 : b + 1, :feat],
                                          in_=st[:, : b + 1 - j0, :])

                aggregate(1, xs, l1_out)

                nc.gpsimd.collective_compute(
                    "AllGather",
                    mybir.AluOpType.bypass,
                    replica_groups=[list(range(NCORES))],
                    ins=[h2sh[:]],
                    outs=[h2full[:]],
                )

                out_stage = {}

                def l2_out(b, ps):
                    tmp = wpool.tile([P, feat], F32, tag="l2tmp")
                    nc.vector.tensor_scalar(
                        out=tmp[:], in0=ps[:],
                        scalar1=dinvc_sb[:, b : b + 1], scalar2=None,
                        op0=mybir.AluOpType.mult,
                    )
                    j0 = (b // 8) * 8
                    if j0 not in out_stage:
                        out_stage[j0] = stpool.tile([P, 8, feat], F32, tag="outst",
                                                    name=f"outst{j0}")
                    st = out_stage[j0]
                    nc.vector.tensor_tensor(
                        out=st[:, b - j0, :], in0=tmp[:], in1=xb2_sb[:, b, :],
                        op=mybir.AluOpType.add,
                    )
                    if b == min(j0 + 7, bpc - 1):
                        nc.sync.dma_start(out=outr[:, j0 : b + 1, :],
                                          in_=st[:, : b + 1 - j0, :])

                aggregate(2, h2full, l2_out)

            for _rep in range(repeat):
                layers()

    nc.compile()
    return nc


# ------------------------------------------------------------------- driver
_CACHE = {}


def _get_compiled(edge_index, n, feat, hid, grp=2):
    key = (hash(edge_index.tobytes()), n, feat, hid, grp)
    if key not in _CACHE:
        plan, percore, dinv_pad = _preprocess(edge_index, n, grp)
        nc = _build_program(plan, feat, hid)
        _CACHE[key] = (plan, percore, dinv_pad, nc)
    return _CACHE[key]


def make_in_maps(plan, percore, dinv_pad, x, W1, b1, W2, b2):
    n, feat = x.shape
    hid = W1.shape[1]
    n_pad, bpc = plan["n_pad"], plan["bpc"]
    nsh = bpc * P

    xs = np.zeros((n_pad, P), ml_dtypes.bfloat16)
    xs[:n, :feat] = (x * dinv_pad[:n, None]).astype(ml_dtypes.bfloat16)
    W1b = W1.astype(ml_dtypes.bfloat16)
    W2b = W2.astype(ml_dtypes.bfloat16)
    b1c = b1.reshape(hid, 1).astype(np.float32)
    ident = np.eye(P, dtype=ml_dtypes.bfloat16)
    xb2 = np.zeros((n_pad, feat), np.float32)
    xb2[:n] = x + b2[None, :]

    in_maps = []
    for c in range(NCORES):
        in_maps.append(dict(
            xs=xs, W1=W1b, b1=b1c, W2=W2b, ident=ident,
            idx16=percore["idx16"][c], s_all=percore["s_all"][c],
            dinv_c=percore["dinv_c"][c],
            xb2=xb2[c * nsh : (c + 1) * nsh],
        ))
    return in_maps


def kernel(x, W1, b1, W2, b2, edge_index):
    x = np.asarray(x, np.float32)
    W1 = np.asarray(W1, np.float32)
    b1 = np.asarray(b1, np.float32)
    W2 = np.asarray(W2, np.float32)
    b2 = np.asarray(b2, np.float32)
    edge_index = np.asarray(edge_index)

    n, feat = x.shape
    hid = W1.shape[1]
    plan, percore, dinv_pad, nc = _get_compiled(edge_index, n, feat, hid)
    nsh = plan["bpc"] * P

    in_maps = make_in_maps(plan, percore, dinv_pad, x, W1, b1, W2, b2)
    res = run_bass_kernel_spmd(nc, in_maps, list(range(NCORES)))
    out = np.concatenate([res.results[c]["out"] for c in range(NCORES)], axis=0)
    return out[:n].astype(np.float32)
